# revision 3
# baseline (speedup 1.0000x reference)
"""DFFN Trainium2 kernel: 1x1 conv -> 2x2 FFT gate -> 3x3 depthwise conv -> gelu-gate -> 1x1 conv.

Data-parallel over batch: 8 NeuronCores, one 192x128x128 image each.

Math:
- The 2x2 rfft2 gate is exactly the per-channel linear map 0.25*H@diag(w)@H on each
  2x2 patch (H = 4x4 Hadamard).  We apply H to x with channel-independent butterflies,
  fold the 0.25*diag(fft_w) scale into four plane-copies of w_in, run the 1x1 conv per
  plane, then apply H again (butterflies) to produce the gated hidden tensor h'.
- Depthwise conv runs on the tensor engine in SIX PSUM-accumulated passes per
  128-output chunk (vs nine naive): the dy=0 and dy=1 tap rows are fused into single
  128-contract matmuls by pairing each 64-slot hidden group with a row-shifted
  physical copy of itself on the other 64 partitions (built by one contiguous
  SBUF->SBUF DMA: a row shift is a +130-element offset).  dy=2 taps run as three
  64-contract passes against the unshifted half.
- Hidden channels are permuted into 512 padded slots (slot 255/511 = zero pad) so each
  128-output dwconv chunk reads one aligned 64-partition slice of one hidden chunk.
- The x~/hidden path runs in bf16 (PE full rate, half the SBUF footprint and PE
  power); in-proj accumulation and everything after the dwconv PSUM is fp32.
  Out-proj runs in float32r (FP22).
"""
import numpy as np
from contextlib import ExitStack

import ml_dtypes
import concourse.bass as bass
import concourse.bacc as bacc
import concourse.tile as tile
from concourse import mybir
from concourse.bass_utils import run_bass_kernel_spmd

F32 = mybir.dt.float32
F32R = mybir.dt.float32r
BF16 = mybir.dt.bfloat16

DIM, HID = 192, 510
H = W = 128
NSLOT = 512
NB = 8          # row bands
PADW = 130      # padded row width
NCORES = 8
OW = [128, 128, 128, 126]   # valid widths of the 4 output-chunk pairs

HID_OF_SLOT = np.full(NSLOT, -1, np.int64)
HID_OF_SLOT[0:255] = np.arange(0, 255)
HID_OF_SLOT[256:511] = np.arange(255, 510)
VALID_SLOT = HID_OF_SLOT >= 0


def band_geom(b):
    # local patch slot lp in [p0, p0+pr) covers global patch row 8b-1+lp
    # local pixel row ly in [0,20) covers image row 16b-2+ly
    p0 = 1 if b == 0 else 0
    pr = 9 if b in (0, NB - 1) else 10
    return p0, pr


def build_module(act="gelu"):
    act_fn = {"gelu": mybir.ActivationFunctionType.Gelu,
              "identity": mybir.ActivationFunctionType.Identity}[act]
    nc = bacc.Bacc()
    x_d = nc.declare_dram_parameter("x", [DIM, H * W], F32, isOutput=False)
    win_d = nc.declare_dram_parameter("w_in4", [2, 96, 4 * NSLOT], BF16, isOutput=False)
    dws_d = nc.declare_dram_parameter("dw_stat", [128, 36 * 128], BF16, isOutput=False)
    wout_d = nc.declare_dram_parameter("w_outT", [128, 4 * 192], F32R, isOutput=False)
    out_d = nc.declare_dram_parameter("out", [DIM, H * W], F32, isOutput=True)

    with tile.TileContext(nc) as tc, ExitStack() as ctx:
        wpool = ctx.enter_context(tc.tile_pool(name="weights", bufs=1))
        xpool = ctx.enter_context(tc.tile_pool(name="xin", bufs=2))
        stpool = ctx.enter_context(tc.tile_pool(name="xstage", bufs=1))
        xtpool = ctx.enter_context(tc.tile_pool(name="xt", bufs=3))
        htpool = ctx.enter_context(tc.tile_pool(name="ht", bufs=2))
        hqpool = ctx.enter_context(tc.tile_pool(name="hstage", bufs=1))
        hppool = ctx.enter_context(tc.tile_pool(name="hpad", bufs=8))
        t1pool = ctx.enter_context(tc.tile_pool(name="t1", bufs=2))
        gpool = ctx.enter_context(tc.tile_pool(name="g", bufs=4))
        opool = ctx.enter_context(tc.tile_pool(name="osb", bufs=1))
        ip_ps = ctx.enter_context(tc.tile_pool(name="ip_ps", bufs=2, space=bass.MemorySpace.PSUM))
        dw_ps = ctx.enter_context(tc.tile_pool(name="dw_ps", bufs=4, space=bass.MemorySpace.PSUM))
        o_ps = ctx.enter_context(tc.tile_pool(name="o_ps", bufs=2, space=bass.MemorySpace.PSUM))

        # ---- weights, loaded once
        win_t = []
        for kc in range(2):
            wt = wpool.tile([96, 4 * NSLOT], BF16, tag=f"win{kc}")
            nc.sync.dma_start(wt[:, :], win_d[kc])
            win_t.append(wt)
        dws_t = wpool.tile([128, 36 * 128], BF16, tag="dws")
        for q in range(4):
            nc.sync.dma_start(dws_t[:, q * 1152:(q + 1) * 1152],
                              dws_d[:, q * 1152:(q + 1) * 1152])
        wout_t = wpool.tile([128, 4 * 192], F32R, tag="wout")
        nc.sync.dma_start(wout_t[:, :], wout_d[:, :])
        zt = wpool.tile([128, 260], F32, tag="zero")
        nc.vector.memset(zt[:, :], 0.0)

        def emit_stage1(b):
            """x band load + forward butterfly -> X~ planes (gpsimd, bf16 out)."""
            p0, pr = band_geom(b)
            npatch = pr * 64
            ys0 = 16 * b - 2 + 2 * p0
            nrow = 2 * pr
            xt_k = []
            for kc in range(2):
                xt_b = xpool.tile([96, 20 * 128], F32, tag="xin")
                nc.sync.dma_start(
                    xt_b[:, 2 * p0 * 128:(2 * p0 + nrow) * 128],
                    x_d[96 * kc:96 * kc + 96, ys0 * 128:(ys0 + nrow) * 128],
                )
                xr = xt_b[:, 256 * p0:256 * (p0 + pr)].rearrange(
                    "p (lp par px o) -> p lp par px o", lp=pr, par=2, px=64, o=2)
                a_even = xr[:, :, :, :, 0]          # [96, pr, 2, 64]
                b_odd = xr[:, :, :, :, 1]
                su = stpool.tile([96, 1280], F32, tag="su")
                tv = stpool.tile([96, 1280], F32, tag="tv")
                su_w = su[:, 0:128 * pr].rearrange("p (lp par px) -> p lp par px", lp=pr, par=2, px=64)
                tv_w = tv[:, 0:128 * pr].rearrange("p (lp par px) -> p lp par px", lp=pr, par=2, px=64)
                nc.gpsimd.tensor_add(su_w, a_even, b_odd)
                nc.gpsimd.tensor_sub(tv_w, a_even, b_odd)
                s_ap = su_w[:, :, 0, :]             # [96, pr, 64]
                u_ap = su_w[:, :, 1, :]
                t_ap = tv_w[:, :, 0, :]
                v_ap = tv_w[:, :, 1, :]
                xt_t = xtpool.tile([96, 4 * 640], BF16, tag="xt")

                def pl(k):
                    return xt_t[:, k * 640:k * 640 + npatch].rearrange(
                        "p (lp px) -> p lp px", lp=pr, px=64)
                nc.gpsimd.tensor_add(pl(0), s_ap, u_ap)
                nc.gpsimd.tensor_add(pl(1), t_ap, v_ap)
                nc.gpsimd.tensor_sub(pl(2), s_ap, u_ap)
                nc.gpsimd.tensor_sub(pl(3), t_ap, v_ap)
                xt_k.append(xt_t)
            return xt_k

        def emit_inproj_mc(b, mc, xt_k):
            """in-proj for one slot-chunk, evict (ACT), inverse butterfly (DVE),
            then DMA row-shifted copies for the paired dwconv passes."""
            p0, pr = band_geom(b)
            npatch = pr * 64
            eng = nc.vector if mc < 2 else nc.gpsimd
            if True:
                ht_t = htpool.tile([128, 4 * 640], F32, tag="ht")
                n0 = npatch // 2
                for k in range(4):
                    for (na, nb_) in ((0, n0), (n0, npatch)):
                        ps = ip_ps.tile([128, 320], F32, tag="ip")
                        nn = nb_ - na
                        for kc in range(2):
                            nc.tensor.matmul(
                                ps[:, 0:nn],
                                win_t[kc][:, k * NSLOT + 128 * mc:k * NSLOT + 128 * (mc + 1)],
                                xt_k[kc][:, k * 640 + na:k * 640 + nb_],
                                start=(kc == 0), stop=(kc == 1),
                            )
                        nc.scalar.copy(ht_t[:, k * 640 + na:k * 640 + nb_], ps[:, 0:nn])

                # AB: parts 0:64 = even group base, parts 64:128 = row-shifted copy
                # CD: parts 64:128 = odd group base, parts 0:64 = row-shifted copy
                ab_t = hppool.tile([128, 20 * PADW], BF16, tag="hpAB")
                cd_t = hppool.tile([128, 20 * PADW], BF16, tag="hpCD")
                ab3 = ab_t[:, :].rearrange("p (ly c) -> p ly c", ly=20, c=130)
                cd3 = cd_t[:, :].rearrange("p (ly c) -> p ly c", ly=20, c=130)
                eng.tensor_copy(ab3[0:64, :, 0], zt[0:64, 0:20])
                eng.tensor_copy(ab3[0:64, :, 129], zt[0:64, 0:20])
                eng.tensor_copy(cd3[64:128, :, 0], zt[64:128, 0:20])
                eng.tensor_copy(cd3[64:128, :, 129], zt[64:128, 0:20])
                if b == 0:
                    eng.tensor_copy(ab_t[0:64, 0:2 * PADW], zt[0:64, :])
                    eng.tensor_copy(cd_t[64:128, 0:2 * PADW], zt[64:128, :])
                if b == NB - 1:
                    eng.tensor_copy(ab_t[0:64, 18 * PADW:20 * PADW], zt[0:64, :])
                    eng.tensor_copy(cd_t[64:128, 18 * PADW:20 * PADW], zt[64:128, :])

                hr = ht_t[:, :].rearrange("p (kp k2 n) -> p kp k2 n", kp=2, k2=2, n=640)
                h02 = hr[:, :, 0, 0:npatch]         # planes 0,2: [128, 2, npatch]
                h13 = hr[:, :, 1, 0:npatch]
                squ = hqpool.tile([128, 1280], F32, tag="squ")
                tqv = hqpool.tile([128, 1280], F32, tag="tqv")
                squ_w = squ[:, :].rearrange("p (k n) -> p k n", k=2)[:, :, 0:npatch]
                tqv_w = tqv[:, :].rearrange("p (k n) -> p k n", k=2)[:, :, 0:npatch]
                eng.tensor_add(squ_w, h02, h13)             # s | u
                eng.tensor_sub(tqv_w, h02, h13)             # t | v
                s_ap = squ_w[:, 0, :].rearrange("p (lp px) -> p lp px", lp=pr, px=64)
                u_ap = squ_w[:, 1, :].rearrange("p (lp px) -> p lp px", lp=pr, px=64)
                t_ap = tqv_w[:, 0, :].rearrange("p (lp px) -> p lp px", lp=pr, px=64)
                v_ap = tqv_w[:, 1, :].rearrange("p (lp px) -> p lp px", lp=pr, px=64)

                def wr_e(iy, ix):
                    r0 = 2 * p0 + iy
                    return ab3[0:64, r0:r0 + 2 * pr - 1:2, 1 + ix:1 + ix + 127:2]

                def wr_o(iy, ix):
                    r0 = 2 * p0 + iy
                    return cd3[64:128, r0:r0 + 2 * pr - 1:2, 1 + ix:1 + ix + 127:2]
                eng.tensor_add(wr_e(0, 0), s_ap[0:64], u_ap[0:64])
                eng.tensor_add(wr_o(0, 0), s_ap[64:128], u_ap[64:128])
                eng.tensor_add(wr_e(0, 1), t_ap[0:64], v_ap[0:64])
                eng.tensor_add(wr_o(0, 1), t_ap[64:128], v_ap[64:128])
                eng.tensor_sub(wr_e(1, 0), s_ap[0:64], u_ap[0:64])
                eng.tensor_sub(wr_o(1, 0), s_ap[64:128], u_ap[64:128])
                eng.tensor_sub(wr_e(1, 1), t_ap[0:64], v_ap[0:64])
                eng.tensor_sub(wr_o(1, 1), t_ap[64:128], v_ap[64:128])

                # row-shifted copies: reading shifted row r == base row r+1
                nc.sync.dma_start(ab_t[64:128, 0:19 * PADW], ab_t[0:64, PADW:20 * PADW])
                nc.sync.dma_start(cd_t[0:64, 0:19 * PADW], cd_t[64:128, PADW:20 * PADW])
            return (ab_t, cd_t)

        def emit_tile(b, tt, hp_mc):
            """dwconv (6 PSUM passes) + gelu-gate + out-proj for one 4-row tile."""
            if True:
                x1ps, x2ps = [], []
                for j in range(8):
                    mc, bh = j // 2, j % 2
                    owj = OW[j % 4]
                    ps = dw_ps.tile([128, 512], F32, tag="dw")
                    t3 = hp_mc[mc][bh][:, :].rearrange("p (ly c) -> p ly c", ly=20, c=130)
                    r0 = 1 + 4 * tt
                    for dx in range(3):
                        rhs = t3[0:128, r0:r0 + 4, dx:dx + 128]
                        lhsT = dws_t[0:128, (j * 3 + dx) * 128:(j * 3 + dx) * 128 + owj]
                        nc.tensor.matmul(ps[0:owj, :], lhsT, rhs,
                                         start=(dx == 0), stop=False)
                    r2 = 3 + 4 * tt
                    for dx in range(3):
                        rhs = t3[64 * bh:64 * bh + 64, r2:r2 + 4, dx:dx + 128]
                        lhsT = dws_t[64 * bh:64 * bh + 64,
                                     3072 + ((j // 2) * 3 + dx) * 128:
                                     3072 + ((j // 2) * 3 + dx) * 128 + owj]
                        nc.tensor.matmul(ps[0:owj, :], lhsT, rhs,
                                         start=False, stop=(dx == 2))
                    (x1ps if j < 4 else x2ps).append((ps, owj))

                g_a = []
                for a in range(4):
                    owa = OW[a]
                    p1, _ = x1ps[a]
                    p2, _ = x2ps[a]
                    t1 = t1pool.tile([128, 512], F32, tag="t1")
                    nc.scalar.activation(t1[0:owa, :], p1[0:owa, :], act_fn)
                    g_t = gpool.tile([128, 512], F32, tag="g")
                    nc.vector.tensor_mul(g_t[0:owa, :].bitcast(F32R), t1[0:owa, :], p2[0:owa, :])
                    g_a.append(g_t)

                osb = opool.tile([96, 1024], F32, tag="osb")
                off = b * 2048 + tt * 512
                for mo in range(2):
                    ops_t = o_ps.tile([96, 512], F32, tag="ops")
                    for a in range(4):
                        kw = OW[a]
                        nc.tensor.matmul(
                            ops_t[:, :],
                            wout_t[0:kw, a * 192 + 96 * mo:a * 192 + 96 * (mo + 1)].bitcast(F32R),
                            g_a[a][0:kw, :].bitcast(F32R),
                            start=(a == 0), stop=(a == 3),
                        )
                    nc.scalar.copy(osb[:, mo * 512:mo * 512 + 512], ops_t[:, :])
                    nc.sync.dma_start(
                        out_d[96 * mo:96 * mo + 96, off:off + 512],
                        osb[:, mo * 512:mo * 512 + 512])

        # ---- software-pipelined band schedule: band b's dwconv tiles are
        # interleaved with band b+1's in-proj quarters so the PE stream
        # never thins out (keeps HAM un-throttled).
        xt_cur = emit_stage1(0)
        hp_cur = [emit_inproj_mc(0, mc, xt_cur) for mc in range(4)]
        for b in range(NB):
            xt_nxt = emit_stage1(b + 1) if b + 1 < NB else None
            hp_nxt = []
            for tt in range(4):
                emit_tile(b, tt, hp_cur)
                if xt_nxt is not None:
                    hp_nxt.append(emit_inproj_mc(b + 1, tt, xt_nxt))
            hp_cur = hp_nxt
    nc.finalize()
    return nc


def prep_weights(w_in, w_dw, fft_w, w_out):
    w_in2 = np.asarray(w_in)[:, :, 0, 0].astype(np.float32)        # [510, 192]
    w_dw2 = np.asarray(w_dw)[:, 0].reshape(2 * HID, 9).astype(np.float32)
    wf = np.asarray(fft_w)[:, 0, 0].reshape(HID, 4).astype(np.float32)
    w_out2 = np.asarray(w_out)[:, :, 0, 0].astype(np.float32)      # [192, 510]

    wslot = np.zeros((NSLOT, DIM), np.float32)
    fslot = np.zeros((NSLOT, 4), np.float32)
    wslot[VALID_SLOT] = w_in2[HID_OF_SLOT[VALID_SLOT]]
    fslot[VALID_SLOT] = wf[HID_OF_SLOT[VALID_SLOT]] * 0.25
    win4 = np.zeros((2, 96, 4 * NSLOT), np.float32)
    for kc in range(2):
        for k in range(4):
            win4[kc, :, k * NSLOT:(k + 1) * NSLOT] = (
                wslot[:, 96 * kc:96 * (kc + 1)] * fslot[:, k:k + 1]).T

    # dwconv stationaries: 24 paired [128,128] (dy0+dy1 fused) then 12 single
    # [64,128] (dy2) packed two chunk-halves per column block.
    dws = np.zeros((128, 36 * 128), np.float32)
    for j in range(8):
        bh = j % 2
        for m in range(OW[j % 4]):
            if j < 4:
                o = 128 * j + m
                slot = o // 2
            else:
                op = 128 * (j - 4) + m
                o = HID + op
                slot = 256 + op // 2
            k = slot - 128 * (j // 2) - 64 * bh
            assert 0 <= k < 64, (j, m, slot, k)
            for dx in range(3):
                col = (j * 3 + dx) * 128 + m
                w0 = w_dw2[o, 0 + dx]       # dy=0
                w1 = w_dw2[o, 3 + dx]       # dy=1
                if bh == 0:                 # AB: base on rows 0:64
                    dws[k, col] = w0
                    dws[64 + k, col] = w1
                else:                       # CD: base on rows 64:128
                    dws[k, col] = w1
                    dws[64 + k, col] = w0
            for dx in range(3):
                col = 3072 + ((j // 2) * 3 + dx) * 128 + m
                dws[64 * bh + k, col] = w_dw2[o, 6 + dx]

    woutT = np.zeros((128, 4 * 192), np.float32)
    for a in range(4):
        kw = OW[a]
        woutT[0:kw, a * 192:(a + 1) * 192] = w_out2[:, 128 * a:128 * a + kw].T
    return (win4.astype(ml_dtypes.bfloat16), dws.astype(ml_dtypes.bfloat16), woutT)


_NC = None


def kernel(x, w_in, w_dw, fft_w, w_out):
    global _NC
    if _NC is None:
        _NC = build_module()
    win4, dws, woutT = prep_weights(w_in, w_dw, fft_w, w_out)
    x = np.ascontiguousarray(np.asarray(x), dtype=np.float32)
    in_maps = [
        {"x": x[i].reshape(DIM, H * W), "w_in4": win4, "dw_stat": dws, "w_outT": woutT}
        for i in range(NCORES)
    ]
    res = run_bass_kernel_spmd(_NC, in_maps, list(range(NCORES)))
    out = np.stack([res.results[i]["out"].reshape(DIM, H, W) for i in range(NCORES)])
    return out.astype(np.float32)


# revision 10
# speedup vs baseline: 1.0653x; 1.0653x over previous
"""DFFN Trainium2 kernel: 1x1 conv -> 2x2 FFT gate -> 3x3 depthwise conv -> gelu-gate -> 1x1 conv.

Data-parallel over batch: 8 NeuronCores, one 192x128x128 image each.

Math:
- The 2x2 rfft2 gate is exactly the per-channel linear map 0.25*H@diag(w)@H on each
  2x2 patch (H = 4x4 Hadamard).  We apply H to x with channel-independent butterflies,
  fold the 0.25*diag(fft_w) scale into four plane-copies of w_in, run the 1x1 conv per
  plane, then apply H again (butterflies) to produce the gated hidden tensor h'.
- Depthwise conv runs on the tensor engine in SIX PSUM-accumulated passes per
  128-output chunk (vs nine naive): the dy=0 and dy=1 tap rows are fused into single
  128-contract matmuls by pairing each 64-slot hidden group with a row-shifted
  physical copy of itself on the other 64 partitions (built by one contiguous
  SBUF->SBUF DMA: a row shift is a +130-element offset).  dy=2 taps run as three
  64-contract passes against the unshifted half.
- Hidden channels are permuted into 512 padded slots (slot 255/511 = zero pad) so each
  128-output dwconv chunk reads one aligned 64-partition slice of one hidden chunk.
- The x~/hidden path runs in bf16 (PE full rate, half the SBUF footprint and PE
  power); in-proj accumulation and everything after the dwconv PSUM is fp32.
  Out-proj runs in float32r (FP22).
"""
import numpy as np
from contextlib import ExitStack

import ml_dtypes
import concourse.bass as bass
import concourse.bacc as bacc
import concourse.tile as tile
from concourse import mybir
from concourse.bass_utils import run_bass_kernel_spmd

F32 = mybir.dt.float32
F32R = mybir.dt.float32r
BF16 = mybir.dt.bfloat16

DIM, HID = 192, 510
H = W = 128
NSLOT = 512
NB = 8          # row bands
PADW = 130      # padded row width
NCORES = 8
OW = [128, 128, 128, 126]   # valid widths of the 4 output-chunk pairs

HID_OF_SLOT = np.full(NSLOT, -1, np.int64)
HID_OF_SLOT[0:255] = np.arange(0, 255)
HID_OF_SLOT[256:511] = np.arange(255, 510)
VALID_SLOT = HID_OF_SLOT >= 0


def band_geom(b):
    # local patch slot lp in [p0, p0+pr) covers global patch row 8b-1+lp
    # local pixel row ly in [0,20) covers image row 16b-2+ly
    p0 = 1 if b == 0 else 0
    pr = 9 if b in (0, NB - 1) else 10
    return p0, pr


def build_module(act="gelu"):
    act_fn = {"gelu": mybir.ActivationFunctionType.Gelu,
              "identity": mybir.ActivationFunctionType.Identity}[act]
    nc = bacc.Bacc()
    x_d = nc.declare_dram_parameter("x", [DIM, H * W], F32, isOutput=False)
    win_d = nc.declare_dram_parameter("w_in4", [2, 96, 4 * NSLOT], BF16, isOutput=False)
    dws_d = nc.declare_dram_parameter("dw_stat", [128, 36 * 128], BF16, isOutput=False)
    wout_d = nc.declare_dram_parameter("w_outT", [128, 4 * 192], F32R, isOutput=False)
    out_d = nc.declare_dram_parameter("out", [DIM, H * W], F32, isOutput=True)

    with tile.TileContext(nc) as tc, ExitStack() as ctx:
        wpool = ctx.enter_context(tc.tile_pool(name="weights", bufs=1))
        xpool = ctx.enter_context(tc.tile_pool(name="xin", bufs=2))
        stpool = ctx.enter_context(tc.tile_pool(name="xstage", bufs=1))
        xtpool = ctx.enter_context(tc.tile_pool(name="xt", bufs=3))
        htpool = ctx.enter_context(tc.tile_pool(name="ht", bufs=2))
        hqpool = ctx.enter_context(tc.tile_pool(name="hstage", bufs=1))
        hppool = ctx.enter_context(tc.tile_pool(name="hpad", bufs=8))
        t1pool = ctx.enter_context(tc.tile_pool(name="t1", bufs=2))
        gpool = ctx.enter_context(tc.tile_pool(name="g", bufs=4))
        opool = ctx.enter_context(tc.tile_pool(name="osb", bufs=1))
        ip_ps = ctx.enter_context(tc.tile_pool(name="ip_ps", bufs=2, space=bass.MemorySpace.PSUM))
        dw_ps = ctx.enter_context(tc.tile_pool(name="dw_ps", bufs=4, space=bass.MemorySpace.PSUM))
        o_ps = ctx.enter_context(tc.tile_pool(name="o_ps", bufs=2, space=bass.MemorySpace.PSUM))

        # ---- weights, loaded once
        win_t = []
        for kc in range(2):
            wt = wpool.tile([96, 4 * NSLOT], BF16, tag=f"win{kc}")
            nc.sync.dma_start(wt[:, :], win_d[kc])
            win_t.append(wt)
        dws_t = wpool.tile([128, 36 * 128], BF16, tag="dws")
        for q in range(4):
            nc.sync.dma_start(dws_t[:, q * 1152:(q + 1) * 1152],
                              dws_d[:, q * 1152:(q + 1) * 1152])
        wout_t = wpool.tile([128, 4 * 192], F32R, tag="wout")
        nc.sync.dma_start(wout_t[:, :], wout_d[:, :])
        zt = wpool.tile([128, 260], F32, tag="zero")
        nc.vector.memset(zt[:, :], 0.0)

        def emit_stage1(b):
            """x band load + forward butterfly -> X~ planes (gpsimd, bf16 out)."""
            p0, pr = band_geom(b)
            npatch = pr * 64
            ys0 = 16 * b - 2 + 2 * p0
            nrow = 2 * pr
            xt_k = []
            for kc in range(2):
                xt_b = xpool.tile([96, 20 * 128], F32, tag="xin")
                nc.sync.dma_start(
                    xt_b[:, 2 * p0 * 128:(2 * p0 + nrow) * 128],
                    x_d[96 * kc:96 * kc + 96, ys0 * 128:(ys0 + nrow) * 128],
                )
                xr = xt_b[:, 256 * p0:256 * (p0 + pr)].rearrange(
                    "p (lp par px o) -> p lp par px o", lp=pr, par=2, px=64, o=2)
                a_even = xr[:, :, :, :, 0]          # [96, pr, 2, 64]
                b_odd = xr[:, :, :, :, 1]
                su = stpool.tile([96, 1280], F32, tag="su")
                tv = stpool.tile([96, 1280], F32, tag="tv")
                su_w = su[:, 0:128 * pr].rearrange("p (lp par px) -> p lp par px", lp=pr, par=2, px=64)
                tv_w = tv[:, 0:128 * pr].rearrange("p (lp par px) -> p lp par px", lp=pr, par=2, px=64)
                nc.gpsimd.tensor_add(su_w, a_even, b_odd)
                nc.gpsimd.tensor_sub(tv_w, a_even, b_odd)
                s_ap = su_w[:, :, 0, :]             # [96, pr, 64]
                u_ap = su_w[:, :, 1, :]
                t_ap = tv_w[:, :, 0, :]
                v_ap = tv_w[:, :, 1, :]
                xt_t = xtpool.tile([96, 4 * 640], BF16, tag="xt")

                def pl(k):
                    return xt_t[:, k * 640:k * 640 + npatch].rearrange(
                        "p (lp px) -> p lp px", lp=pr, px=64)
                nc.gpsimd.tensor_add(pl(0), s_ap, u_ap)
                nc.gpsimd.tensor_add(pl(1), t_ap, v_ap)
                nc.gpsimd.tensor_sub(pl(2), s_ap, u_ap)
                nc.gpsimd.tensor_sub(pl(3), t_ap, v_ap)
                xt_k.append(xt_t)
            return xt_k

        def emit_inproj_mc(b, mc, xt_k):
            """in-proj for one slot-chunk, evict (ACT), inverse butterfly (DVE),
            then DMA row-shifted copies for the paired dwconv passes.

            Hidden rows use a deinterleaved column layout so the butterfly
            writes are contiguous (DVE packed-bf16 mode): cols 0..63 = even
            pixels 0..126, col 64 = right pad (pixel 128), col 65 = left pad
            (pixel -1), cols 66..129 = odd pixels 1..127."""
            p0, pr = band_geom(b)
            npatch = pr * 64
            eng = nc.vector
            if True:
                ht_t = htpool.tile([128, 4 * 640], F32, tag="ht")
                n0 = npatch // 2
                for k in range(4):
                    for (na, nb_) in ((0, n0), (n0, npatch)):
                        ps = ip_ps.tile([128, 320], F32, tag="ip")
                        nn = nb_ - na
                        for kc in range(2):
                            nc.tensor.matmul(
                                ps[:, 0:nn],
                                win_t[kc][:, k * NSLOT + 128 * mc:k * NSLOT + 128 * (mc + 1)],
                                xt_k[kc][:, k * 640 + na:k * 640 + nb_],
                                start=(kc == 0), stop=(kc == 1),
                            )
                        nc.scalar.copy(ht_t[:, k * 640 + na:k * 640 + nb_], ps[:, 0:nn])

                # AB: parts 0:64 = even group base, parts 64:128 = row-shifted copy
                # CD: parts 64:128 = odd group base, parts 0:64 = row-shifted copy
                ab_t = hppool.tile([128, 20 * PADW], BF16, tag="hpAB")
                cd_t = hppool.tile([128, 20 * PADW], BF16, tag="hpCD")
                ab3 = ab_t[:, :].rearrange("p (ly c) -> p ly c", ly=20, c=130)
                cd3 = cd_t[:, :].rearrange("p (ly c) -> p ly c", ly=20, c=130)
                eng.tensor_copy(ab3[0:64, :, 64:66], zt[0:64, 0:40].rearrange(
                    "p (a b) -> p a b", a=20, b=2))
                eng.tensor_copy(cd3[64:128, :, 64:66], zt[64:128, 0:40].rearrange(
                    "p (a b) -> p a b", a=20, b=2))
                if b == 0:
                    eng.tensor_copy(ab_t[0:64, 0:2 * PADW], zt[0:64, :])
                    eng.tensor_copy(cd_t[64:128, 0:2 * PADW], zt[64:128, :])
                if b == NB - 1:
                    eng.tensor_copy(ab_t[0:64, 18 * PADW:20 * PADW], zt[0:64, :])
                    eng.tensor_copy(cd_t[64:128, 18 * PADW:20 * PADW], zt[64:128, :])

                hr = ht_t[:, :].rearrange("p (kp k2 n) -> p kp k2 n", kp=2, k2=2, n=640)
                h02 = hr[:, :, 0, 0:npatch]         # planes 0,2: [128, 2, npatch]
                h13 = hr[:, :, 1, 0:npatch]
                squ = hqpool.tile([128, 1280], F32, tag="squ")
                tqv = hqpool.tile([128, 1280], F32, tag="tqv")
                squ_w = squ[:, :].rearrange("p (k n) -> p k n", k=2)[:, :, 0:npatch]
                tqv_w = tqv[:, :].rearrange("p (k n) -> p k n", k=2)[:, :, 0:npatch]
                eng.tensor_add(squ_w, h02, h13)             # s | u
                eng.tensor_sub(tqv_w, h02, h13)             # t | v
                s_ap = squ_w[:, 0, :].rearrange("p (lp px) -> p lp px", lp=pr, px=64)
                u_ap = squ_w[:, 1, :].rearrange("p (lp px) -> p lp px", lp=pr, px=64)
                t_ap = tqv_w[:, 0, :].rearrange("p (lp px) -> p lp px", lp=pr, px=64)
                v_ap = tqv_w[:, 1, :].rearrange("p (lp px) -> p lp px", lp=pr, px=64)

                def wr_e(iy, ix):
                    r0 = 2 * p0 + iy
                    cb = 0 if ix == 0 else 66
                    return ab3[0:64, r0:r0 + 2 * pr - 1:2, cb:cb + 64]

                def wr_o(iy, ix):
                    r0 = 2 * p0 + iy
                    cb = 0 if ix == 0 else 66
                    return cd3[64:128, r0:r0 + 2 * pr - 1:2, cb:cb + 64]
                eng.tensor_add(wr_e(0, 0), s_ap[0:64], u_ap[0:64])
                eng.tensor_add(wr_o(0, 0), s_ap[64:128], u_ap[64:128])
                eng.tensor_add(wr_e(0, 1), t_ap[0:64], v_ap[0:64])
                eng.tensor_add(wr_o(0, 1), t_ap[64:128], v_ap[64:128])
                eng.tensor_sub(wr_e(1, 0), s_ap[0:64], u_ap[0:64])
                eng.tensor_sub(wr_o(1, 0), s_ap[64:128], u_ap[64:128])
                eng.tensor_sub(wr_e(1, 1), t_ap[0:64], v_ap[0:64])
                eng.tensor_sub(wr_o(1, 1), t_ap[64:128], v_ap[64:128])

                # row-shifted copies: reading shifted row r == base row r+1
                nc.sync.dma_start(ab_t[64:128, 0:19 * PADW], ab_t[0:64, PADW:20 * PADW])
                nc.sync.dma_start(cd_t[0:64, 0:19 * PADW], cd_t[64:128, PADW:20 * PADW])
            return (ab_t, cd_t)

        def emit_tile(b, tt, hp_mc):
            """dwconv (6 PSUM passes) + gelu-gate + out-proj for one 4-row tile."""
            if True:
                # PSUM columns are in sigma order: cols 0..255 = odd pixels
                # 1,3..127 of the 4 rows, cols 256..511 = even pixels 0..126.
                # Taps dx=0/2 read both deinterleave planes in one 4D rhs;
                # dx=1 needs the opposite plane order, so it is split into two
                # contiguous half-width matmuls (same stationary).
                x1ps, x2ps = [], []
                for j in range(8):
                    mc, bh = j // 2, j % 2
                    owj = OW[j % 4]
                    ps = dw_ps.tile([128, 512], F32, tag="dw")
                    t3 = hp_mc[mc][bh][:, :].rearrange("p (ly c) -> p ly c", ly=20, c=130)

                    def passes(parts0, parts1, r, cols, first, last):
                        v = t3[parts0:parts1, r:r + 4, :].rearrange(
                            "p r (pl k) -> p pl r k", pl=2, k=65)
                        lhs0 = dws_t[parts0:parts1, cols + 0 * 128:cols + 0 * 128 + owj]
                        lhs1 = dws_t[parts0:parts1, cols + 1 * 128:cols + 1 * 128 + owj]
                        lhs2 = dws_t[parts0:parts1, cols + 2 * 128:cols + 2 * 128 + owj]
                        nc.tensor.matmul(ps[0:owj, :], lhs0, v[:, :, :, 0:64],
                                         start=first, stop=False)
                        nc.tensor.matmul(ps[0:owj, :], lhs2, v[:, :, :, 1:65],
                                         start=False, stop=False)
                        nc.tensor.matmul(ps[0:owj, 0:256], lhs1, v[:, 1, :, 1:65],
                                         start=False, stop=False)
                        nc.tensor.matmul(ps[0:owj, 256:512], lhs1, v[:, 0, :, 0:64],
                                         start=False, stop=last)

                    passes(0, 128, 1 + 4 * tt, (j * 3) * 128, True, False)
                    passes(64 * bh, 64 * bh + 64, 3 + 4 * tt,
                           3072 + ((j // 2) * 3) * 128, False, True)
                    (x1ps if j < 4 else x2ps).append((ps, owj))

                g_a = []
                for a in range(4):
                    owa = OW[a]
                    p1, _ = x1ps[a]
                    p2, _ = x2ps[a]
                    t1 = t1pool.tile([128, 512], F32, tag="t1")
                    nc.scalar.activation(t1[0:owa, :], p1[0:owa, :], act_fn)
                    g_t = gpool.tile([128, 512], F32, tag="g")
                    nc.vector.tensor_mul(g_t[0:owa, :].bitcast(F32R), t1[0:owa, :], p2[0:owa, :])
                    g_a.append(g_t)

                osb = opool.tile([96, 1024], F32, tag="osb")
                off = b * 2048 + tt * 512
                for mo in range(2):
                    ops_t = o_ps.tile([96, 512], F32, tag="ops")
                    for a in range(4):
                        kw = OW[a]
                        nc.tensor.matmul(
                            ops_t[:, :],
                            wout_t[0:kw, a * 192 + 96 * mo:a * 192 + 96 * (mo + 1)].bitcast(F32R),
                            g_a[a][0:kw, :].bitcast(F32R),
                            start=(a == 0), stop=(a == 3),
                        )
                    ov = ops_t[:, :].rearrange("p (pl r i) -> p pl r i", pl=2, r=4, i=64)
                    db = osb[:, mo * 512:mo * 512 + 512].rearrange(
                        "p (r i pl) -> p r i pl", r=4, i=64, pl=2)
                    nc.scalar.copy(db[:, :, :, 1], ov[:, 0, :, :])   # odd pixels
                    nc.scalar.copy(db[:, :, :, 0], ov[:, 1, :, :])   # even pixels
                    nc.sync.dma_start(
                        out_d[96 * mo:96 * mo + 96, off:off + 512],
                        osb[:, mo * 512:mo * 512 + 512])

        # ---- software-pipelined band schedule: band b's dwconv tiles are
        # interleaved with band b+1's in-proj quarters so the PE stream
        # never thins out (keeps HAM un-throttled).
        xt_cur = emit_stage1(0)
        hp_cur = [emit_inproj_mc(0, mc, xt_cur) for mc in range(4)]
        for b in range(NB):
            xt_nxt = emit_stage1(b + 1) if b + 1 < NB else None
            hp_nxt = []
            for tt in range(4):
                emit_tile(b, tt, hp_cur)
                if xt_nxt is not None:
                    hp_nxt.append(emit_inproj_mc(b + 1, tt, xt_nxt))
            hp_cur = hp_nxt
    nc.finalize()
    return nc


def prep_weights(w_in, w_dw, fft_w, w_out):
    w_in2 = np.asarray(w_in)[:, :, 0, 0].astype(np.float32)        # [510, 192]
    w_dw2 = np.asarray(w_dw)[:, 0].reshape(2 * HID, 9).astype(np.float32)
    wf = np.asarray(fft_w)[:, 0, 0].reshape(HID, 4).astype(np.float32)
    w_out2 = np.asarray(w_out)[:, :, 0, 0].astype(np.float32)      # [192, 510]

    wslot = np.zeros((NSLOT, DIM), np.float32)
    fslot = np.zeros((NSLOT, 4), np.float32)
    wslot[VALID_SLOT] = w_in2[HID_OF_SLOT[VALID_SLOT]]
    fslot[VALID_SLOT] = wf[HID_OF_SLOT[VALID_SLOT]] * 0.25
    win4 = np.zeros((2, 96, 4 * NSLOT), np.float32)
    for kc in range(2):
        for k in range(4):
            win4[kc, :, k * NSLOT:(k + 1) * NSLOT] = (
                wslot[:, 96 * kc:96 * (kc + 1)] * fslot[:, k:k + 1]).T

    # dwconv stationaries: 24 paired [128,128] (dy0+dy1 fused) then 12 single
    # [64,128] (dy2) packed two chunk-halves per column block.
    dws = np.zeros((128, 36 * 128), np.float32)
    for j in range(8):
        bh = j % 2
        for m in range(OW[j % 4]):
            if j < 4:
                o = 128 * j + m
                slot = o // 2
            else:
                op = 128 * (j - 4) + m
                o = HID + op
                slot = 256 + op // 2
            k = slot - 128 * (j // 2) - 64 * bh
            assert 0 <= k < 64, (j, m, slot, k)
            for dx in range(3):
                col = (j * 3 + dx) * 128 + m
                w0 = w_dw2[o, 0 + dx]       # dy=0
                w1 = w_dw2[o, 3 + dx]       # dy=1
                if bh == 0:                 # AB: base on rows 0:64
                    dws[k, col] = w0
                    dws[64 + k, col] = w1
                else:                       # CD: base on rows 64:128
                    dws[k, col] = w1
                    dws[64 + k, col] = w0
            for dx in range(3):
                col = 3072 + ((j // 2) * 3 + dx) * 128 + m
                dws[64 * bh + k, col] = w_dw2[o, 6 + dx]

    woutT = np.zeros((128, 4 * 192), np.float32)
    for a in range(4):
        kw = OW[a]
        woutT[0:kw, a * 192:(a + 1) * 192] = w_out2[:, 128 * a:128 * a + kw].T
    return (win4.astype(ml_dtypes.bfloat16), dws.astype(ml_dtypes.bfloat16), woutT)


_NC = None


def kernel(x, w_in, w_dw, fft_w, w_out):
    global _NC
    if _NC is None:
        _NC = build_module()
    win4, dws, woutT = prep_weights(w_in, w_dw, fft_w, w_out)
    x = np.ascontiguousarray(np.asarray(x), dtype=np.float32)
    in_maps = [
        {"x": x[i].reshape(DIM, H * W), "w_in4": win4, "dw_stat": dws, "w_outT": woutT}
        for i in range(NCORES)
    ]
    res = run_bass_kernel_spmd(_NC, in_maps, list(range(NCORES)))
    out = np.stack([res.results[i]["out"].reshape(DIM, H, W) for i in range(NCORES)])
    return out.astype(np.float32)


# revision 18
# speedup vs baseline: 1.1636x; 1.0922x over previous
"""DFFN Trainium2 kernel: 1x1 conv -> 2x2 FFT gate -> 3x3 depthwise conv -> gelu-gate -> 1x1 conv.

Data-parallel over batch: 8 NeuronCores, one 192x128x128 image each.

Math:
- The 2x2 rfft2 gate is exactly the per-channel linear map 0.25*H@diag(w)@H on each
  2x2 patch (H = 4x4 Hadamard).  We apply H to x with channel-independent butterflies,
  fold the 0.25*diag(fft_w) scale into four plane-copies of w_in, run the 1x1 conv per
  plane, then apply H again (butterflies) to produce the gated hidden tensor h'.
- Depthwise conv runs on the tensor engine in SIX PSUM-accumulated passes per
  128-output chunk (vs nine naive): the dy=0 and dy=1 tap rows are fused into single
  128-contract matmuls by pairing each 64-slot hidden group with a row-shifted
  physical copy of itself on the other 64 partitions (built by one contiguous
  SBUF->SBUF DMA: a row shift is a +130-element offset).  dy=2 taps run as three
  64-contract passes against the unshifted half.
- Hidden channels are permuted into 512 padded slots (slot 255/511 = zero pad) so each
  128-output dwconv chunk reads one aligned 64-partition slice of one hidden chunk.
- The x~/hidden path runs in bf16 (PE full rate, half the SBUF footprint and PE
  power); in-proj accumulation and everything after the dwconv PSUM is fp32.
  Out-proj runs in float32r (FP22).
"""
import numpy as np
from contextlib import ExitStack

import ml_dtypes
import concourse.bass as bass
import concourse.bacc as bacc
import concourse.tile as tile
from concourse import mybir
from concourse.bass_utils import run_bass_kernel_spmd

F32 = mybir.dt.float32
F32R = mybir.dt.float32r
BF16 = mybir.dt.bfloat16

DIM, HID = 192, 510
H = W = 128
NSLOT = 512
NB = 8          # row bands
PADW = 130      # padded row width
NCORES = 8
OW = [128, 128, 128, 126]   # valid widths of the 4 output-chunk pairs

HID_OF_SLOT = np.full(NSLOT, -1, np.int64)
HID_OF_SLOT[0:255] = np.arange(0, 255)
HID_OF_SLOT[256:511] = np.arange(255, 510)
VALID_SLOT = HID_OF_SLOT >= 0


def band_geom(b):
    # local patch slot lp in [p0, p0+pr) covers global patch row 8b-1+lp
    # local pixel row ly in [0,20) covers image row 16b-2+ly
    p0 = 1 if b == 0 else 0
    pr = 9 if b in (0, NB - 1) else 10
    return p0, pr


def build_module(act="gelu"):
    act_fn = {"gelu": mybir.ActivationFunctionType.Gelu,
              "identity": mybir.ActivationFunctionType.Identity}[act]
    nc = bacc.Bacc()
    x_d = nc.declare_dram_parameter("x", [DIM, H * W], F32, isOutput=False)
    win_d = nc.declare_dram_parameter("w_in4", [2, 96, 4 * NSLOT], BF16, isOutput=False)
    dws_d = nc.declare_dram_parameter("dw_stat", [128, 36 * 128], BF16, isOutput=False)
    wout_d = nc.declare_dram_parameter("w_outT", [128, 4 * 192], BF16, isOutput=False)
    out_d = nc.declare_dram_parameter("out", [DIM, H * W], F32, isOutput=True)

    with tile.TileContext(nc) as tc, ExitStack() as ctx:
        wpool = ctx.enter_context(tc.tile_pool(name="weights", bufs=1))
        xpool = ctx.enter_context(tc.tile_pool(name="xin", bufs=2))
        stpool = ctx.enter_context(tc.tile_pool(name="xstage", bufs=1))
        xtpool = ctx.enter_context(tc.tile_pool(name="xt", bufs=3))
        htpool = ctx.enter_context(tc.tile_pool(name="ht", bufs=2))
        hqpool = ctx.enter_context(tc.tile_pool(name="hstage", bufs=1))
        hppool = ctx.enter_context(tc.tile_pool(name="hpad", bufs=8))
        tstage = ctx.enter_context(tc.tile_pool(name="hpT", bufs=3))
        t1pool = ctx.enter_context(tc.tile_pool(name="t1", bufs=2))
        gpool = ctx.enter_context(tc.tile_pool(name="g", bufs=4))
        opool = ctx.enter_context(tc.tile_pool(name="osb", bufs=1))
        ip_ps = ctx.enter_context(tc.tile_pool(name="ip_ps", bufs=2, space=bass.MemorySpace.PSUM))
        dw_ps = ctx.enter_context(tc.tile_pool(name="dw_ps", bufs=4, space=bass.MemorySpace.PSUM))
        o_ps = ctx.enter_context(tc.tile_pool(name="o_ps", bufs=2, space=bass.MemorySpace.PSUM))

        # ---- weights, loaded once
        win_t = []
        for kc in range(2):
            wt = wpool.tile([96, 4 * NSLOT], BF16, tag=f"win{kc}")
            nc.sync.dma_start(wt[:, :], win_d[kc])
            win_t.append(wt)
        dws_t = wpool.tile([128, 36 * 128], BF16, tag="dws")
        for q in range(4):
            nc.sync.dma_start(dws_t[:, q * 1152:(q + 1) * 1152],
                              dws_d[:, q * 1152:(q + 1) * 1152])
        wout_t = wpool.tile([128, 4 * 192], BF16, tag="wout")
        nc.sync.dma_start(wout_t[:, :], wout_d[:, :])
        zt = wpool.tile([128, 260], F32, tag="zero")
        nc.vector.memset(zt[:, :], 0.0)

        def emit_stage1(b):
            """x band load + forward butterfly -> X~ planes (gpsimd, bf16 out)."""
            p0, pr = band_geom(b)
            npatch = pr * 64
            ys0 = 16 * b - 2 + 2 * p0
            nrow = 2 * pr
            xt_k = []
            for kc in range(2):
                xt_b = xpool.tile([96, 20 * 128], F32, tag="xin")
                nc.sync.dma_start(
                    xt_b[:, 2 * p0 * 128:(2 * p0 + nrow) * 128],
                    x_d[96 * kc:96 * kc + 96, ys0 * 128:(ys0 + nrow) * 128],
                )
                xr = xt_b[:, 256 * p0:256 * (p0 + pr)].rearrange(
                    "p (lp par px o) -> p lp par px o", lp=pr, par=2, px=64, o=2)
                a_even = xr[:, :, :, :, 0]          # [96, pr, 2, 64]
                b_odd = xr[:, :, :, :, 1]
                su = stpool.tile([96, 1280], F32, tag="su")
                tv = stpool.tile([96, 1280], F32, tag="tv")
                su_w = su[:, 0:128 * pr].rearrange("p (lp par px) -> p lp par px", lp=pr, par=2, px=64)
                tv_w = tv[:, 0:128 * pr].rearrange("p (lp par px) -> p lp par px", lp=pr, par=2, px=64)
                nc.gpsimd.tensor_add(su_w, a_even, b_odd)
                nc.gpsimd.tensor_sub(tv_w, a_even, b_odd)
                s_ap = su_w[:, :, 0, :]             # [96, pr, 64]
                u_ap = su_w[:, :, 1, :]
                t_ap = tv_w[:, :, 0, :]
                v_ap = tv_w[:, :, 1, :]
                xt_t = xtpool.tile([96, 4 * 640], BF16, tag="xt")

                def pl(k):
                    return xt_t[:, k * 640:k * 640 + npatch].rearrange(
                        "p (lp px) -> p lp px", lp=pr, px=64)
                nc.gpsimd.tensor_add(pl(0), s_ap, u_ap)
                nc.gpsimd.tensor_add(pl(1), t_ap, v_ap)
                nc.gpsimd.tensor_sub(pl(2), s_ap, u_ap)
                nc.gpsimd.tensor_sub(pl(3), t_ap, v_ap)
                xt_k.append(xt_t)
            return xt_k

        def emit_inproj_mc(b, mc, xt_k):
            """in-proj for one slot-chunk, evict (ACT), inverse butterfly (DVE),
            then DMA row-shifted copies for the paired dwconv passes.

            Hidden rows use a deinterleaved column layout so the butterfly
            writes are contiguous (DVE packed-bf16 mode): cols 0..63 = even
            pixels 0..126, col 64 = right pad (pixel 128), col 65 = left pad
            (pixel -1), cols 66..129 = odd pixels 1..127."""
            p0, pr = band_geom(b)
            npatch = pr * 64
            eng = nc.vector
            if True:
                ht_t = htpool.tile([128, 4 * 640], F32, tag="ht")
                n0 = npatch // 2
                for k in range(4):
                    for (na, nb_) in ((0, n0), (n0, npatch)):
                        ps = ip_ps.tile([128, 320], F32, tag="ip")
                        nn = nb_ - na
                        for kc in range(2):
                            nc.tensor.matmul(
                                ps[:, 0:nn],
                                win_t[kc][:, k * NSLOT + 128 * mc:k * NSLOT + 128 * (mc + 1)],
                                xt_k[kc][:, k * 640 + na:k * 640 + nb_],
                                start=(kc == 0), stop=(kc == 1),
                            )
                        nc.scalar.copy(ht_t[:, k * 640 + na:k * 640 + nb_], ps[:, 0:nn])

                # T: parts 0:64 = even group base, 64:128 = odd group base.
                # Butterflies write T once (full 128 partitions); DMA then
                # builds AB = [even base | even shifted] and
                # CD = [odd shifted | odd base] for the paired dwconv passes.
                t_t = tstage.tile([128, 20 * PADW], BF16, tag="hpT")
                ab_t = hppool.tile([128, 20 * PADW], BF16, tag="hpAB")
                cd_t = hppool.tile([128, 20 * PADW], BF16, tag="hpCD")
                t3s = t_t[:, :].rearrange("p (ly c) -> p ly c", ly=20, c=130)
                eng.tensor_copy(t3s[:, :, 64:66], zt[:, 0:40].rearrange(
                    "p (a b) -> p a b", a=20, b=2))
                if b == 0:
                    eng.tensor_copy(t_t[:, 0:2 * PADW], zt[:, :])
                if b == NB - 1:
                    eng.tensor_copy(t_t[:, 18 * PADW:20 * PADW], zt[:, :])

                hr = ht_t[:, :].rearrange("p (kp k2 n) -> p kp k2 n", kp=2, k2=2, n=640)
                h02 = hr[:, :, 0, 0:npatch]         # planes 0,2: [128, 2, npatch]
                h13 = hr[:, :, 1, 0:npatch]
                squ = hqpool.tile([128, 1280], F32, tag="squ")
                tqv = hqpool.tile([128, 1280], F32, tag="tqv")
                squ_w = squ[:, :].rearrange("p (k n) -> p k n", k=2)[:, :, 0:npatch]
                tqv_w = tqv[:, :].rearrange("p (k n) -> p k n", k=2)[:, :, 0:npatch]
                eng.tensor_add(squ_w, h02, h13)             # s | u
                eng.tensor_sub(tqv_w, h02, h13)             # t | v
                s_ap = squ_w[:, 0, :].rearrange("p (lp px) -> p lp px", lp=pr, px=64)
                u_ap = squ_w[:, 1, :].rearrange("p (lp px) -> p lp px", lp=pr, px=64)
                t_ap = tqv_w[:, 0, :].rearrange("p (lp px) -> p lp px", lp=pr, px=64)
                v_ap = tqv_w[:, 1, :].rearrange("p (lp px) -> p lp px", lp=pr, px=64)

                def wr(iy, ix):
                    r0 = 2 * p0 + iy
                    cb = 0 if ix == 0 else 66
                    return t3s[:, r0:r0 + 2 * pr - 1:2, cb:cb + 64]
                eng.tensor_add(wr(0, 0), s_ap, u_ap)
                eng.tensor_add(wr(0, 1), t_ap, v_ap)
                eng.tensor_sub(wr(1, 0), s_ap, u_ap)
                eng.tensor_sub(wr(1, 1), t_ap, v_ap)

                # base copies + row-shifted copies (reading shifted row r ==
                # base row r+1); DMA is the only engine that crosses partitions
                nc.sync.dma_start(ab_t[0:64, :], t_t[0:64, :])
                nc.sync.dma_start(ab_t[64:128, 0:19 * PADW], t_t[0:64, PADW:20 * PADW])
                nc.sync.dma_start(cd_t[64:128, :], t_t[64:128, :])
                nc.sync.dma_start(cd_t[0:64, 0:19 * PADW], t_t[64:128, PADW:20 * PADW])
            return (ab_t, cd_t)

        def emit_tile(b, tt, hp_mc):
            """dwconv (6 PSUM passes) + gelu-gate + out-proj for one 4-row tile."""
            if True:
                # PSUM columns are in sigma order: cols 0..255 = odd pixels
                # 1,3..127 of the 4 rows, cols 256..511 = even pixels 0..126.
                # Taps dx=0/2 read both deinterleave planes in one 4D rhs;
                # dx=1 needs the opposite plane order, so it is split into two
                # contiguous half-width matmuls (same stationary).
                x1ps, x2ps = [], []
                for j in range(8):
                    mc, bh = j // 2, j % 2
                    owj = OW[j % 4]
                    ps = dw_ps.tile([128, 512], F32, tag="dw")
                    t3 = hp_mc[mc][bh][:, :].rearrange("p (ly c) -> p ly c", ly=20, c=130)

                    def passes(parts0, parts1, r, cols, first, last):
                        v = t3[parts0:parts1, r:r + 4, :].rearrange(
                            "p r (pl k) -> p pl r k", pl=2, k=65)
                        lhs0 = dws_t[parts0:parts1, cols + 0 * 128:cols + 0 * 128 + owj]
                        lhs1 = dws_t[parts0:parts1, cols + 1 * 128:cols + 1 * 128 + owj]
                        lhs2 = dws_t[parts0:parts1, cols + 2 * 128:cols + 2 * 128 + owj]
                        nc.tensor.matmul(ps[0:owj, :], lhs0, v[:, :, :, 0:64],
                                         start=first, stop=False)
                        nc.tensor.matmul(ps[0:owj, :], lhs2, v[:, :, :, 1:65],
                                         start=False, stop=False)
                        nc.tensor.matmul(ps[0:owj, 0:256], lhs1, v[:, 1, :, 1:65],
                                         start=False, stop=False)
                        nc.tensor.matmul(ps[0:owj, 256:512], lhs1, v[:, 0, :, 0:64],
                                         start=False, stop=last)

                    passes(0, 128, 1 + 4 * tt, (j * 3) * 128, True, False)
                    passes(64 * bh, 64 * bh + 64, 3 + 4 * tt,
                           3072 + ((j // 2) * 3) * 128, False, True)
                    (x1ps if j < 4 else x2ps).append((ps, owj))

                g_a = []
                for a in range(4):
                    owa = OW[a]
                    p1, _ = x1ps[a]
                    p2, _ = x2ps[a]
                    t1 = t1pool.tile([128, 512], F32, tag="t1")
                    nc.scalar.activation(t1[0:owa, :], p1[0:owa, :], act_fn)
                    g_t = gpool.tile([128, 512], BF16, tag="g")
                    nc.vector.tensor_mul(g_t[0:owa, :], t1[0:owa, :], p2[0:owa, :])
                    g_a.append(g_t)

                osb = opool.tile([96, 1024], F32, tag="osb")
                off = b * 2048 + tt * 512
                for mo in range(2):
                    ops_t = o_ps.tile([96, 512], F32, tag="ops")
                    for a in range(4):
                        kw = OW[a]
                        nc.tensor.matmul(
                            ops_t[:, :],
                            wout_t[0:kw, a * 192 + 96 * mo:a * 192 + 96 * (mo + 1)],
                            g_a[a][0:kw, :],
                            start=(a == 0), stop=(a == 3),
                        )
                    ov = ops_t[:, :].rearrange("p (pl r i) -> p pl r i", pl=2, r=4, i=64)
                    db = osb[:, mo * 512:mo * 512 + 512].rearrange(
                        "p (r i pl) -> p r i pl", r=4, i=64, pl=2)
                    nc.scalar.copy(db[:, :, :, 1], ov[:, 0, :, :])   # odd pixels
                    nc.scalar.copy(db[:, :, :, 0], ov[:, 1, :, :])   # even pixels
                    nc.sync.dma_start(
                        out_d[96 * mo:96 * mo + 96, off:off + 512],
                        osb[:, mo * 512:mo * 512 + 512])

        # ---- software-pipelined band schedule: band b's dwconv tiles are
        # interleaved with band b+1's in-proj quarters so the PE stream
        # never thins out (keeps HAM un-throttled).
        xt_cur = emit_stage1(0)
        hp_cur = [emit_inproj_mc(0, mc, xt_cur) for mc in range(4)]
        for b in range(NB):
            xt_nxt = emit_stage1(b + 1) if b + 1 < NB else None
            hp_nxt = []
            for tt in range(4):
                emit_tile(b, tt, hp_cur)
                if xt_nxt is not None:
                    hp_nxt.append(emit_inproj_mc(b + 1, tt, xt_nxt))
            hp_cur = hp_nxt
    nc.finalize()
    return nc


def prep_weights(w_in, w_dw, fft_w, w_out):
    w_in2 = np.asarray(w_in)[:, :, 0, 0].astype(np.float32)        # [510, 192]
    w_dw2 = np.asarray(w_dw)[:, 0].reshape(2 * HID, 9).astype(np.float32)
    wf = np.asarray(fft_w)[:, 0, 0].reshape(HID, 4).astype(np.float32)
    w_out2 = np.asarray(w_out)[:, :, 0, 0].astype(np.float32)      # [192, 510]

    wslot = np.zeros((NSLOT, DIM), np.float32)
    fslot = np.zeros((NSLOT, 4), np.float32)
    wslot[VALID_SLOT] = w_in2[HID_OF_SLOT[VALID_SLOT]]
    fslot[VALID_SLOT] = wf[HID_OF_SLOT[VALID_SLOT]] * 0.25
    win4 = np.zeros((2, 96, 4 * NSLOT), np.float32)
    for kc in range(2):
        for k in range(4):
            win4[kc, :, k * NSLOT:(k + 1) * NSLOT] = (
                wslot[:, 96 * kc:96 * (kc + 1)] * fslot[:, k:k + 1]).T

    # dwconv stationaries: 24 paired [128,128] (dy0+dy1 fused) then 12 single
    # [64,128] (dy2) packed two chunk-halves per column block.
    dws = np.zeros((128, 36 * 128), np.float32)
    for j in range(8):
        bh = j % 2
        for m in range(OW[j % 4]):
            if j < 4:
                o = 128 * j + m
                slot = o // 2
            else:
                op = 128 * (j - 4) + m
                o = HID + op
                slot = 256 + op // 2
            k = slot - 128 * (j // 2) - 64 * bh
            assert 0 <= k < 64, (j, m, slot, k)
            for dx in range(3):
                col = (j * 3 + dx) * 128 + m
                w0 = w_dw2[o, 0 + dx]       # dy=0
                w1 = w_dw2[o, 3 + dx]       # dy=1
                if bh == 0:                 # AB: base on rows 0:64
                    dws[k, col] = w0
                    dws[64 + k, col] = w1
                else:                       # CD: base on rows 64:128
                    dws[k, col] = w1
                    dws[64 + k, col] = w0
            for dx in range(3):
                col = 3072 + ((j // 2) * 3 + dx) * 128 + m
                dws[64 * bh + k, col] = w_dw2[o, 6 + dx]

    woutT = np.zeros((128, 4 * 192), np.float32)
    for a in range(4):
        kw = OW[a]
        woutT[0:kw, a * 192:(a + 1) * 192] = w_out2[:, 128 * a:128 * a + kw].T
    return (win4.astype(ml_dtypes.bfloat16), dws.astype(ml_dtypes.bfloat16),
            woutT.astype(ml_dtypes.bfloat16))


_NC = None


def kernel(x, w_in, w_dw, fft_w, w_out):
    global _NC
    if _NC is None:
        _NC = build_module()
    win4, dws, woutT = prep_weights(w_in, w_dw, fft_w, w_out)
    x = np.ascontiguousarray(np.asarray(x), dtype=np.float32)
    in_maps = [
        {"x": x[i].reshape(DIM, H * W), "w_in4": win4, "dw_stat": dws, "w_outT": woutT}
        for i in range(NCORES)
    ]
    res = run_bass_kernel_spmd(_NC, in_maps, list(range(NCORES)))
    out = np.stack([res.results[i]["out"].reshape(DIM, H, W) for i in range(NCORES)])
    return out.astype(np.float32)


# revision 20
# speedup vs baseline: 1.2858x; 1.1051x over previous
"""DFFN Trainium2 kernel: 1x1 conv -> 2x2 FFT gate -> 3x3 depthwise conv -> gelu-gate -> 1x1 conv.

Data-parallel over batch: 8 NeuronCores, one 192x128x128 image each.

Math:
- The 2x2 rfft2 gate is exactly the per-channel linear map 0.25*H@diag(w)@H on each
  2x2 patch (H = 4x4 Hadamard).  We apply H to x with channel-independent butterflies,
  fold the 0.25*diag(fft_w) scale into four plane-copies of w_in, run the 1x1 conv per
  plane, then apply H again (butterflies) to produce the gated hidden tensor h'.
- Depthwise conv runs on the tensor engine in SIX PSUM-accumulated passes per
  128-output chunk (vs nine naive): the dy=0 and dy=1 tap rows are fused into single
  128-contract matmuls by pairing each 64-slot hidden group with a row-shifted
  physical copy of itself on the other 64 partitions (built by one contiguous
  SBUF->SBUF DMA: a row shift is a +130-element offset).  dy=2 taps run as three
  64-contract passes against the unshifted half.
- Hidden channels are permuted into 512 padded slots (slot 255/511 = zero pad) so each
  128-output dwconv chunk reads one aligned 64-partition slice of one hidden chunk.
- The x~/hidden path runs in bf16 (PE full rate, half the SBUF footprint and PE
  power); in-proj accumulation and everything after the dwconv PSUM is fp32.
  Out-proj runs in float32r (FP22).
"""
import numpy as np
from contextlib import ExitStack

import ml_dtypes
import concourse.bass as bass
import concourse.bacc as bacc
import concourse.tile as tile
from concourse import mybir
from concourse.bass_utils import run_bass_kernel_spmd

F32 = mybir.dt.float32
F32R = mybir.dt.float32r
BF16 = mybir.dt.bfloat16

DIM, HID = 192, 510
H = W = 128
NSLOT = 512
NB = 8          # row bands
PADW = 130      # padded row width
NCORES = 8
OW = [128, 128, 128, 126]   # valid widths of the 4 output-chunk pairs

HID_OF_SLOT = np.full(NSLOT, -1, np.int64)
HID_OF_SLOT[0:255] = np.arange(0, 255)
HID_OF_SLOT[256:511] = np.arange(255, 510)
VALID_SLOT = HID_OF_SLOT >= 0


def band_geom(b):
    # local patch slot lp in [p0, p0+pr) covers global patch row 8b-1+lp
    # local pixel row ly in [0,20) covers image row 16b-2+ly
    p0 = 1 if b == 0 else 0
    pr = 9 if b in (0, NB - 1) else 10
    return p0, pr


def build_module(act="gelu"):
    act_fn = {"gelu": mybir.ActivationFunctionType.Gelu,
              "identity": mybir.ActivationFunctionType.Identity}[act]
    nc = bacc.Bacc()
    x_d = nc.declare_dram_parameter("x", [DIM, H * W], F32, isOutput=False)
    win_d = nc.declare_dram_parameter("w_in4", [2, 96, 4 * NSLOT], BF16, isOutput=False)
    dws_d = nc.declare_dram_parameter("dw_stat", [128, 36 * 128], BF16, isOutput=False)
    wout_d = nc.declare_dram_parameter("w_outT", [128, 4 * 192], BF16, isOutput=False)
    out_d = nc.declare_dram_parameter("out", [DIM, H * W], F32, isOutput=True)

    with tile.TileContext(nc) as tc, ExitStack() as ctx:
        wpool = ctx.enter_context(tc.tile_pool(name="weights", bufs=1))
        xpool = ctx.enter_context(tc.tile_pool(name="xin", bufs=2))
        stpool = ctx.enter_context(tc.tile_pool(name="xstage", bufs=1))
        xtpool = ctx.enter_context(tc.tile_pool(name="xt", bufs=3))
        htpool = ctx.enter_context(tc.tile_pool(name="ht", bufs=2))
        hqpool = ctx.enter_context(tc.tile_pool(name="hstage", bufs=1))
        hppool = ctx.enter_context(tc.tile_pool(name="hpad", bufs=8))
        tstage = ctx.enter_context(tc.tile_pool(name="hpT", bufs=3))
        t1pool = ctx.enter_context(tc.tile_pool(name="t1", bufs=2))
        gpool = ctx.enter_context(tc.tile_pool(name="g", bufs=4))
        opool = ctx.enter_context(tc.tile_pool(name="osb", bufs=1))
        ip_ps = ctx.enter_context(tc.tile_pool(name="ip_ps", bufs=2, space=bass.MemorySpace.PSUM))
        dw_ps = ctx.enter_context(tc.tile_pool(name="dw_ps", bufs=4, space=bass.MemorySpace.PSUM))
        o_ps = ctx.enter_context(tc.tile_pool(name="o_ps", bufs=2, space=bass.MemorySpace.PSUM))

        # ---- weights, loaded once
        win_t = []
        for kc in range(2):
            wt = wpool.tile([96, 4 * NSLOT], BF16, tag=f"win{kc}")
            nc.sync.dma_start(wt[:, :], win_d[kc])
            win_t.append(wt)
        dws_t = wpool.tile([128, 36 * 128], BF16, tag="dws")
        for q in range(4):
            nc.sync.dma_start(dws_t[:, q * 1152:(q + 1) * 1152],
                              dws_d[:, q * 1152:(q + 1) * 1152])
        wout_t = wpool.tile([128, 4 * 192], BF16, tag="wout")
        nc.sync.dma_start(wout_t[:, :], wout_d[:, :])
        zt = wpool.tile([128, 260], F32, tag="zero")
        nc.vector.memset(zt[:, :], 0.0)

        def emit_stage1(b):
            """x band load + forward butterfly -> X~ planes (gpsimd, bf16 out)."""
            p0, pr = band_geom(b)
            npatch = pr * 64
            ys0 = 16 * b - 2 + 2 * p0
            nrow = 2 * pr
            xt_k = []
            for kc in range(2):
                xt_b = xpool.tile([96, 20 * 128], F32, tag="xin")
                nc.sync.dma_start(
                    xt_b[:, 2 * p0 * 128:(2 * p0 + nrow) * 128],
                    x_d[96 * kc:96 * kc + 96, ys0 * 128:(ys0 + nrow) * 128],
                )
                xr = xt_b[:, 256 * p0:256 * (p0 + pr)].rearrange(
                    "p (lp par px o) -> p lp par px o", lp=pr, par=2, px=64, o=2)
                a_even = xr[:, :, :, :, 0]          # [96, pr, 2, 64]
                b_odd = xr[:, :, :, :, 1]
                su = stpool.tile([96, 1280], F32, tag="su")
                tv = stpool.tile([96, 1280], F32, tag="tv")
                su_w = su[:, 0:128 * pr].rearrange("p (lp par px) -> p lp par px", lp=pr, par=2, px=64)
                tv_w = tv[:, 0:128 * pr].rearrange("p (lp par px) -> p lp par px", lp=pr, par=2, px=64)
                nc.gpsimd.tensor_add(su_w, a_even, b_odd)
                nc.gpsimd.tensor_sub(tv_w, a_even, b_odd)
                s_ap = su_w[:, :, 0, :]             # [96, pr, 64]
                u_ap = su_w[:, :, 1, :]
                t_ap = tv_w[:, :, 0, :]
                v_ap = tv_w[:, :, 1, :]
                xt_t = xtpool.tile([96, 4 * 640], BF16, tag="xt")

                def pl(k):
                    return xt_t[:, k * 640:k * 640 + npatch].rearrange(
                        "p (lp px) -> p lp px", lp=pr, px=64)
                nc.gpsimd.tensor_add(pl(0), s_ap, u_ap)
                nc.gpsimd.tensor_add(pl(1), t_ap, v_ap)
                nc.gpsimd.tensor_sub(pl(2), s_ap, u_ap)
                nc.gpsimd.tensor_sub(pl(3), t_ap, v_ap)
                xt_k.append(xt_t)
            return xt_k

        def emit_inproj_mc(b, mc, xt_k):
            """in-proj for one slot-chunk, evict (ACT), inverse butterfly (DVE),
            then DMA row-shifted copies for the paired dwconv passes.

            Hidden rows use a deinterleaved column layout so the butterfly
            writes are contiguous (DVE packed-bf16 mode): cols 0..63 = even
            pixels 0..126, col 64 = right pad (pixel 128), col 65 = left pad
            (pixel -1), cols 66..129 = odd pixels 1..127."""
            p0, pr = band_geom(b)
            npatch = pr * 64
            eng = nc.vector
            if True:
                ht_t = htpool.tile([128, 4 * 640], F32, tag="ht")
                n0 = npatch // 2
                for k in range(4):
                    for (na, nb_) in ((0, n0), (n0, npatch)):
                        ps = ip_ps.tile([128, 320], F32, tag="ip")
                        nn = nb_ - na
                        for kc in range(2):
                            nc.tensor.matmul(
                                ps[:, 0:nn],
                                win_t[kc][:, k * NSLOT + 128 * mc:k * NSLOT + 128 * (mc + 1)],
                                xt_k[kc][:, k * 640 + na:k * 640 + nb_],
                                start=(kc == 0), stop=(kc == 1),
                            )
                        nc.scalar.copy(ht_t[:, k * 640 + na:k * 640 + nb_], ps[:, 0:nn])

                # T: parts 0:64 = even group base, 64:128 = odd group base.
                # Butterflies write T once (full 128 partitions); DMA then
                # builds AB = [even base | even shifted] and
                # CD = [odd shifted | odd base] for the paired dwconv passes.
                t_t = tstage.tile([128, 20 * PADW], BF16, tag="hpT")
                ab_t = hppool.tile([128, 20 * PADW], BF16, tag="hpAB")
                cd_t = hppool.tile([128, 20 * PADW], BF16, tag="hpCD")
                t3s = t_t[:, :].rearrange("p (ly c) -> p ly c", ly=20, c=130)
                eng.tensor_copy(t3s[:, :, 64:66], zt[:, 0:40].rearrange(
                    "p (a b) -> p a b", a=20, b=2))
                if b == 0:
                    eng.tensor_copy(t_t[:, 0:2 * PADW], zt[:, :])
                if b == NB - 1:
                    eng.tensor_copy(t_t[:, 18 * PADW:20 * PADW], zt[:, :])

                hr = ht_t[:, :].rearrange("p (kp k2 n) -> p kp k2 n", kp=2, k2=2, n=640)
                h02 = hr[:, :, 0, 0:npatch]         # planes 0,2: [128, 2, npatch]
                h13 = hr[:, :, 1, 0:npatch]
                squ = hqpool.tile([128, 1280], F32, tag="squ")
                tqv = hqpool.tile([128, 1280], F32, tag="tqv")
                squ_w = squ[:, :].rearrange("p (k n) -> p k n", k=2)[:, :, 0:npatch]
                tqv_w = tqv[:, :].rearrange("p (k n) -> p k n", k=2)[:, :, 0:npatch]
                eng.tensor_add(squ_w, h02, h13)             # s | u
                eng.tensor_sub(tqv_w, h02, h13)             # t | v
                s_ap = squ_w[:, 0, :].rearrange("p (lp px) -> p lp px", lp=pr, px=64)
                u_ap = squ_w[:, 1, :].rearrange("p (lp px) -> p lp px", lp=pr, px=64)
                t_ap = tqv_w[:, 0, :].rearrange("p (lp px) -> p lp px", lp=pr, px=64)
                v_ap = tqv_w[:, 1, :].rearrange("p (lp px) -> p lp px", lp=pr, px=64)

                def wr(iy, ix):
                    r0 = 2 * p0 + iy
                    cb = 0 if ix == 0 else 66
                    return t3s[:, r0:r0 + 2 * pr - 1:2, cb:cb + 64]
                eng.tensor_add(wr(0, 0), s_ap, u_ap)
                eng.tensor_add(wr(0, 1), t_ap, v_ap)
                eng.tensor_sub(wr(1, 0), s_ap, u_ap)
                eng.tensor_sub(wr(1, 1), t_ap, v_ap)

                # base copies + row-shifted copies (reading shifted row r ==
                # base row r+1); DMA is the only engine that crosses partitions
                nc.sync.dma_start(ab_t[0:64, :], t_t[0:64, :])
                nc.sync.dma_start(ab_t[64:128, 0:19 * PADW], t_t[0:64, PADW:20 * PADW])
                nc.sync.dma_start(cd_t[64:128, :], t_t[64:128, :])
                nc.sync.dma_start(cd_t[0:64, 0:19 * PADW], t_t[64:128, PADW:20 * PADW])
            return (ab_t, cd_t)

        def emit_tile(b, tt, hp_mc):
            """dwconv (6 PSUM passes) + gelu-gate + out-proj for one 4-row tile."""
            if True:
                # PSUM columns are in sigma order: cols 0..255 = odd pixels
                # 1,3..127 of the 4 rows, cols 256..511 = even pixels 0..126.
                # Taps dx=0/2 read both deinterleave planes in one 4D rhs;
                # dx=1 needs the opposite plane order, so it is split into two
                # contiguous half-width matmuls (same stationary).
                def dw_chunk(j):
                    mc, bh = j // 2, j % 2
                    owj = OW[j % 4]
                    ps = dw_ps.tile([128, 512], F32, tag="dw")
                    t3 = hp_mc[mc][bh][:, :].rearrange("p (ly c) -> p ly c", ly=20, c=130)

                    def passes(parts0, parts1, r, cols, first, last):
                        v = t3[parts0:parts1, r:r + 4, :].rearrange(
                            "p r (pl k) -> p pl r k", pl=2, k=65)
                        lhs0 = dws_t[parts0:parts1, cols + 0 * 128:cols + 0 * 128 + owj]
                        lhs1 = dws_t[parts0:parts1, cols + 1 * 128:cols + 1 * 128 + owj]
                        lhs2 = dws_t[parts0:parts1, cols + 2 * 128:cols + 2 * 128 + owj]
                        nc.tensor.matmul(ps[0:owj, :], lhs0, v[:, :, :, 0:64],
                                         start=first, stop=False)
                        nc.tensor.matmul(ps[0:owj, :], lhs2, v[:, :, :, 1:65],
                                         start=False, stop=False)
                        nc.tensor.matmul(ps[0:owj, 0:256], lhs1, v[:, 1, :, 1:65],
                                         start=False, stop=False)
                        nc.tensor.matmul(ps[0:owj, 256:512], lhs1, v[:, 0, :, 0:64],
                                         start=False, stop=last)

                    passes(0, 128, 1 + 4 * tt, (j * 3) * 128, True, False)
                    passes(64 * bh, 64 * bh + 64, 3 + 4 * tt,
                           3072 + ((j // 2) * 3) * 128, False, True)
                    return ps

                # per pair (x1_a, x2_a): matmuls then immediate gelu-gate, so
                # PSUM banks recycle incrementally and ACT/DVE work overlaps
                # the next chunk's matmuls instead of bunching at tile end.
                g_a = []
                for a in range(4):
                    owa = OW[a]
                    p1 = dw_chunk(a)
                    p2 = dw_chunk(a + 4)
                    t1 = t1pool.tile([128, 512], F32, tag="t1")
                    nc.scalar.activation(t1[0:owa, :], p1[0:owa, :], act_fn)
                    g_t = gpool.tile([128, 512], BF16, tag="g")
                    nc.vector.tensor_mul(g_t[0:owa, :], t1[0:owa, :], p2[0:owa, :])
                    g_a.append(g_t)

                osb = opool.tile([96, 1024], F32, tag="osb")
                off = b * 2048 + tt * 512
                for mo in range(2):
                    ops_t = o_ps.tile([96, 512], F32, tag="ops")
                    for a in range(4):
                        kw = OW[a]
                        nc.tensor.matmul(
                            ops_t[:, :],
                            wout_t[0:kw, a * 192 + 96 * mo:a * 192 + 96 * (mo + 1)],
                            g_a[a][0:kw, :],
                            start=(a == 0), stop=(a == 3),
                        )
                    ov = ops_t[:, :].rearrange("p (pl r i) -> p pl r i", pl=2, r=4, i=64)
                    db = osb[:, mo * 512:mo * 512 + 512].rearrange(
                        "p (r i pl) -> p r i pl", r=4, i=64, pl=2)
                    nc.scalar.copy(db[:, :, :, 1], ov[:, 0, :, :])   # odd pixels
                    nc.scalar.copy(db[:, :, :, 0], ov[:, 1, :, :])   # even pixels
                    nc.sync.dma_start(
                        out_d[96 * mo:96 * mo + 96, off:off + 512],
                        osb[:, mo * 512:mo * 512 + 512])

        # ---- software-pipelined band schedule: band b's dwconv tiles are
        # interleaved with band b+1's in-proj quarters so the PE stream
        # never thins out (keeps HAM un-throttled).
        xt_cur = emit_stage1(0)
        hp_cur = [emit_inproj_mc(0, mc, xt_cur) for mc in range(4)]
        for b in range(NB):
            xt_nxt = emit_stage1(b + 1) if b + 1 < NB else None
            hp_nxt = []
            for tt in range(4):
                if xt_nxt is not None:
                    hp_nxt.append(emit_inproj_mc(b + 1, tt, xt_nxt))
                emit_tile(b, tt, hp_cur)
            hp_cur = hp_nxt
    nc.finalize()
    return nc


def prep_weights(w_in, w_dw, fft_w, w_out):
    w_in2 = np.asarray(w_in)[:, :, 0, 0].astype(np.float32)        # [510, 192]
    w_dw2 = np.asarray(w_dw)[:, 0].reshape(2 * HID, 9).astype(np.float32)
    wf = np.asarray(fft_w)[:, 0, 0].reshape(HID, 4).astype(np.float32)
    w_out2 = np.asarray(w_out)[:, :, 0, 0].astype(np.float32)      # [192, 510]

    wslot = np.zeros((NSLOT, DIM), np.float32)
    fslot = np.zeros((NSLOT, 4), np.float32)
    wslot[VALID_SLOT] = w_in2[HID_OF_SLOT[VALID_SLOT]]
    fslot[VALID_SLOT] = wf[HID_OF_SLOT[VALID_SLOT]] * 0.25
    win4 = np.zeros((2, 96, 4 * NSLOT), np.float32)
    for kc in range(2):
        for k in range(4):
            win4[kc, :, k * NSLOT:(k + 1) * NSLOT] = (
                wslot[:, 96 * kc:96 * (kc + 1)] * fslot[:, k:k + 1]).T

    # dwconv stationaries: 24 paired [128,128] (dy0+dy1 fused) then 12 single
    # [64,128] (dy2) packed two chunk-halves per column block.
    dws = np.zeros((128, 36 * 128), np.float32)
    for j in range(8):
        bh = j % 2
        for m in range(OW[j % 4]):
            if j < 4:
                o = 128 * j + m
                slot = o // 2
            else:
                op = 128 * (j - 4) + m
                o = HID + op
                slot = 256 + op // 2
            k = slot - 128 * (j // 2) - 64 * bh
            assert 0 <= k < 64, (j, m, slot, k)
            for dx in range(3):
                col = (j * 3 + dx) * 128 + m
                w0 = w_dw2[o, 0 + dx]       # dy=0
                w1 = w_dw2[o, 3 + dx]       # dy=1
                if bh == 0:                 # AB: base on rows 0:64
                    dws[k, col] = w0
                    dws[64 + k, col] = w1
                else:                       # CD: base on rows 64:128
                    dws[k, col] = w1
                    dws[64 + k, col] = w0
            for dx in range(3):
                col = 3072 + ((j // 2) * 3 + dx) * 128 + m
                dws[64 * bh + k, col] = w_dw2[o, 6 + dx]

    woutT = np.zeros((128, 4 * 192), np.float32)
    for a in range(4):
        kw = OW[a]
        woutT[0:kw, a * 192:(a + 1) * 192] = w_out2[:, 128 * a:128 * a + kw].T
    return (win4.astype(ml_dtypes.bfloat16), dws.astype(ml_dtypes.bfloat16),
            woutT.astype(ml_dtypes.bfloat16))


_NC = None


def kernel(x, w_in, w_dw, fft_w, w_out):
    global _NC
    if _NC is None:
        _NC = build_module()
    win4, dws, woutT = prep_weights(w_in, w_dw, fft_w, w_out)
    x = np.ascontiguousarray(np.asarray(x), dtype=np.float32)
    in_maps = [
        {"x": x[i].reshape(DIM, H * W), "w_in4": win4, "dw_stat": dws, "w_outT": woutT}
        for i in range(NCORES)
    ]
    res = run_bass_kernel_spmd(_NC, in_maps, list(range(NCORES)))
    out = np.stack([res.results[i]["out"].reshape(DIM, H, W) for i in range(NCORES)])
    return out.astype(np.float32)
